# revision 27
# baseline (speedup 1.0000x reference)
"""Trainium2 Bass kernel for nn_DiffusionModel_56822417326086.

Causal multi-head self-attention block:
    qkv = x @ w_qkv ; split into 8 heads of 64
    e = (q @ k^T) * DH^-0.5 ; causal + key-padding mask ; a = softmax(e)
    o = a @ v ; y = o @ w_out + b_out ; y *= m

Sharding (8 cores, zero collectives):
    core c -> batch b = c // 2, head-quad q = c % 2 (heads 4q..4q+3).
    Each core computes q/k/v for its 4 heads over its whole batch, full
    causal attention for those heads, and the partial output projection
    y_partial = o[heads] @ w_out[head rows].  Host sums the two partials
    per batch (linear unshard), adds b_out, applies the query-side mask.

On-device layout notes:
  - scores are computed TRANSPOSED: sT[key, query] so that the A@V
    contraction (over keys) has keys on the partition dim.
  - softmax denominators come for free as a 65th "ones" column of V.
  - no max-subtraction in softmax: scores are O(1) here, exp is safe.
  - ALL matmuls are bf16 (fp32 "HIGH" mode streams ~1.5x slower) except
    the tiny K=1 recip-broadcast matmuls (f32r).
  - all matmul operands live at partition base 0 (base-64 operands fault
    on this runtime), so q/k are stored per-head at partitions 0-63.
  - all 4 heads of one key block share a 2-bank PSUM tile [128, 1024]
    so one ACT Exp op covers them (ACT per-op overhead is ~250 ns).
  - The kernel is ScalarE(exp)-bound: emission order interleaves the
    qkv projection and the per-chunk normalize+output-projection tail
    INTO the attention loop so PE/DVE/DMA work hides under ACT.
  - diag score blocks are query-trimmed: scores/exp/A@V skip queries
    below the diagonal; only the 128x128 diagonal triangle is masked.
"""

import numpy as np
import ml_dtypes
from contextlib import ExitStack

B, T, D, H = 4, 2048, 512, 8
DH = D // H
SCALE = DH ** -0.5
QC = 512           # query-chunk (free dim of score matmuls)
NQC = T // QC      # 4
KB = 128           # key-block (partition dim of score tiles)

_CACHE = {}


def _build_program():
    import concourse.mybir as mybir
    import concourse.tile as tile
    from concourse import bacc

    f32 = mybir.dt.float32
    f32r = mybir.dt.float32r
    bf16 = mybir.dt.bfloat16
    Exp = mybir.ActivationFunctionType.Exp

    nc = bacc.Bacc("TRN2", target_bir_lowering=False, debug=False)

    xT_d = nc.dram_tensor("xT", [D, T], bf16, kind="ExternalInput").ap()
    wq_d = nc.dram_tensor("wq2", [2, D, 128], bf16, kind="ExternalInput").ap()
    wk_d = nc.dram_tensor("wk2", [2, D, 128], bf16, kind="ExternalInput").ap()
    wv_d = nc.dram_tensor("wv4", [D, 256], bf16, kind="ExternalInput").ap()
    wo_d = nc.dram_tensor("wo4", [256, D], bf16, kind="ExternalInput").ap()
    dm_d = nc.dram_tensor("dmS", [128, 128], bf16, kind="ExternalInput").ap()
    mk_d = nc.dram_tensor("mkey", [T, 1], f32, kind="ExternalInput").ap()
    y_d = nc.dram_tensor("y", [T, D], f32, kind="ExternalOutput").ap()

    with tile.TileContext(nc) as tc, ExitStack() as ctx:
        consts = ctx.enter_context(tc.tile_pool(name="consts", bufs=1))
        work = ctx.enter_context(tc.tile_pool(name="work", bufs=2))
        sm_pool = ctx.enter_context(tc.tile_pool(name="sm", bufs=3))
        exp_pool = ctx.enter_context(tc.tile_pool(name="exp", bufs=6))
        ps_big = ctx.enter_context(tc.tile_pool(name="psb", bufs=3, space="PSUM"))
        ps_o = ctx.enter_context(tc.tile_pool(name="pso", bufs=1, space="PSUM"))

        # ---- persistent tiles ----------------------------------------------
        # packed q^T/k^T: partitions 0-63 = head A of pair, 64-127 = head B
        qT2 = consts.tile([128, 2, T], bf16)
        kT2 = consts.tile([128, 2, T], bf16)
        vsb = consts.tile([128, 16, 4, 65], bf16)
        wo = consts.tile([128, 2, D], bf16)
        mk = consts.tile([128, 16], f32)
        ones41 = consts.tile([128, 4, 1], f32)
        oUA = consts.tile([64, 2, T], bf16)
        oUB = consts.tile([64, 2, T], bf16)
        oTn2 = consts.tile([128, 2, T], bf16)
        dmS = consts.tile([128, 128], bf16)
        ones64 = consts.tile([1, 64], f32)
        ones64b = consts.tile([1, 64], bf16)
        wq = consts.tile([128, 2, 4, 128], bf16)
        wk = consts.tile([128, 2, 4, 128], bf16)
        wv = consts.tile([128, 4, 256], bf16)
        xT = consts.tile([128, 4, T], bf16)

        nc.vector.memset(ones41[:], 1.0)
        nc.vector.memset(ones64[:], 1.0)
        nc.vector.memset(ones64b[:], 1.0)

        # first x column + qk weights land first so real matmuls can
        # start (and keep HAM warm) as early as possible
        _eng = [nc.sync, nc.gpsimd, nc.scalar, nc.gpsimd]
        for kc in range(4):
            _eng[kc].dma_start(xT[:, kc, 0:512], xT_d[kc * 128:(kc + 1) * 128, 0:512])
        for p in range(2):
            for kc in range(4):
                nc.gpsimd.dma_start(wq[:, p, kc, :],
                                    wq_d[p, kc * 128:(kc + 1) * 128, :])
                nc.scalar.dma_start(wk[:, p, kc, :],
                                    wk_d[p, kc * 128:(kc + 1) * 128, :])
        for kc in range(4):
            nc.scalar.dma_start(wv[:, kc, :], wv_d[kc * 128:(kc + 1) * 128, :])
        nc.sync.dma_start(dmS[:], dm_d)
        for p in range(2):
            nc.sync.dma_start(wo[:, p, :], wo_d[p * 128:(p + 1) * 128, :])
        for rc in range(16):
            nc.gpsimd.dma_start(mk[:, rc:rc + 1], mk_d[rc * 128:(rc + 1) * 128, :])
        for rc4 in range(1, 4):
            for kc in range(4):
                _eng[kc].dma_start(
                    xT[:, kc, rc4 * 512:(rc4 + 1) * 512],
                    xT_d[kc * 128:(kc + 1) * 128, rc4 * 512:(rc4 + 1) * 512])

        # warmup: get HAM to K=8/8 while DMAs land; also trigger the exp
        # ACT table load (~2.7us) before the first real score tile.
        warm = consts.tile([1, 512], bf16)
        nc.vector.memset(warm[:], 1.0)
        wex = consts.tile([1, 512], bf16)
        for i in range(20):
            wps = ps_big.tile([64, 512], f32, tag="scores")
            nc.tensor.matmul(wps[:], warm[0:1, 0:64], warm[:],
                             start=True, stop=True)
            if i == 0:
                nc.scalar.activation(wex[:], wps[0:1, :], Exp, scale=0.001)

        # ---- emission helpers ----------------------------------------------
        def emit_qk(p, rc4):
            sl = slice(rc4 * 512, (rc4 + 1) * 512)
            pqk = ps_big.tile([128, 2, 512], f32, tag="scores")
            for kc in range(4):
                nc.tensor.matmul(pqk[:, 0, :], wq[:, p, kc, :], xT[:, kc, sl],
                                 start=kc == 0, stop=kc == 3)
                nc.tensor.matmul(pqk[:, 1, :], wk[:, p, kc, :], xT[:, kc, sl],
                                 start=kc == 0, stop=kc == 3)
            nc.vector.tensor_copy(qT2[:, p, sl], pqk[:, 0, :])
            nc.vector.tensor_copy(kT2[:, p, sl], pqk[:, 1, :])

        def emit_v_rc(rc):
            psv = ps_big.tile([128, 4, 64], f32, tag="scores")
            for kc in range(4):
                nc.tensor.matmul(psv[:], xT[:, kc, rc * 128:(rc + 1) * 128],
                                 wv[:, kc, :], start=kc == 0, stop=kc == 3)
            nc.vector.tensor_scalar_mul(vsb[:, rc, :, 0:64], psv[:],
                                        mk[:, rc:rc + 1])
            nc.vector.tensor_scalar_mul(vsb[:, rc, :, 64:65], ones41[:],
                                        mk[:, rc:rc + 1])

        def emit_v(rc4):
            for rc in range(4 * rc4, 4 * rc4 + 4):
                emit_v_rc(rc)

        def emit_av(item, oA, oB, nkb):
            """Deferred A@V accumulations for one key block (one pair)."""
            kb, ex, t0 = item
            nc.tensor.matmul(oA[0:65, t0:512], vsb[:, kb, 2 * cur_p[0], :],
                             ex[:, t0:512], start=kb == 0, stop=kb == nkb - 1)
            nc.tensor.matmul(oB[0:65, t0:512], vsb[:, kb, 2 * cur_p[0] + 1, :],
                             ex[:, 512:1024 - t0], start=kb == 0,
                             stop=kb == nkb - 1)

        cur_p = [0]

        def emit_attn(p, qc, inject=None):
            """Attention for (pair p, query chunk qc).  `inject` is a list of
            closures emitted mid-loop (qkv bursts / previous chunk's tail) so
            other engines' work lands inside ACT's busy window."""
            cur_p[0] = p
            nkb = 4 * (qc + 1)
            qbase = qc * QC
            oA = ps_o.tile([128, 512], f32, tag="oA")
            oB = ps_o.tile([128, 512], f32, tag="oB")
            avq = []
            inj = list(inject or [])
            for kb in range(nkb):
                ksl = slice(kb * KB, (kb + 1) * KB)
                v = kb - (nkb - 4)          # >= 0 on the 4 diagonal blocks
                t0 = 128 * v if v > 0 else 0  # trim: queries < t0 are below diag
                sps = ps_big.tile([128, 1024], f32, tag="scores")
                # row-tiled pair: K=64 each, concurrent in the array;
                # outputs land in DIFFERENT PSUM banks (same-bank
                # dual-write faults the exec unit)
                # head B's trimmed queries pack adjacent to head A's so one
                # contiguous exp op covers exactly the valid region
                qsl = slice(qbase + t0, qbase + 512)
                nc.tensor.matmul(sps[:, t0:512], kT2[0:64, p, ksl],
                                 qT2[0:64, p, qsl], start=True, stop=True,
                                 tile_position=(0, 0))
                nc.tensor.matmul(sps[:, 512:1024 - t0], kT2[64:128, p, ksl],
                                 qT2[64:128, p, qsl], start=True, stop=True,
                                 tile_position=(64, 0))
                ex = exp_pool.tile([128, 1024], bf16, tag="exp")
                nc.scalar.activation(ex[:, t0:1024 - t0], sps[:, t0:1024 - t0],
                                     Exp, scale=SCALE)
                if v >= 0:
                    # strict-upper triangle of the 128x128 diagonal sub-block
                    nc.vector.tensor_mul(ex[:, t0:t0 + 128],
                                         ex[:, t0:t0 + 128], dmS[:])
                    nc.vector.tensor_mul(ex[:, 512:640], ex[:, 512:640],
                                         dmS[:])
                avq.append((kb, ex, t0))
                if len(avq) > 1:
                    emit_av(avq.pop(0), oA, oB, nkb)
                if inj and kb >= 2:
                    inj.pop(0)()
            for fn in inj:
                fn()
            while avq:
                emit_av(avq.pop(0), oA, oB, nkb)

            # evict o from PSUM (DVE reads at most one PSUM operand per op,
            # so the normalize multiply needs o in SBUF)
            qsl = slice(qbase, qbase + 512)
            nc.vector.tensor_copy(oUA[:, p, qsl], oA[0:64, :])
            nc.vector.tensor_copy(oUB[:, p, qsl], oB[0:64, :])
            # denominators: stage the PSUM ones-row into SBUF, DMA it down
            # to partition 0, and only then reciprocal + cast -- custom DVE
            # ops (reciprocal_approx_fast) corrupt on hardware unless their
            # operands sit at base partition 0 in SBUF
            scr = work.tile([65, 1024], f32, tag="sumscr")
            nc.vector.tensor_copy(scr[64:65, 0:512], oA[64:65, :])
            nc.vector.tensor_copy(scr[64:65, 512:1024], oB[64:65, :])
            sums = sm_pool.tile([1, 1024], f32, tag="sums")
            nc.sync.dma_start(sums[:], scr[64:65, :])
            rec_f = sm_pool.tile([1, 1024], f32, tag="recf")
            nc.vector.reciprocal_approx_fast(rec_f[0:1, 0:512],
                                             sums[0:1, 0:512])
            nc.vector.reciprocal_approx_fast(rec_f[0:1, 512:1024],
                                             sums[0:1, 512:1024])
            rec_b = sm_pool.tile([1, 1024], bf16, tag="recb")
            nc.vector.tensor_copy(rec_b[:], rec_f[:])
            return rec_b

        def emit_norm(p, qc, rec_b):
            """Broadcast 1/sum over the 64 dh partitions and scale o."""
            qsl = slice(qc * QC, (qc + 1) * QC)
            bc = ps_big.tile([64, 2, 512], f32, tag="scores")
            nc.tensor.matmul(bc[:, 0, :], ones64b[:], rec_b[0:1, 0:512],
                             start=True, stop=True)
            nc.tensor.matmul(bc[:, 1, :], ones64b[:], rec_b[0:1, 512:1024],
                             start=True, stop=True)
            nc.vector.tensor_mul(oTn2[0:64, p, qsl], oUA[:, p, qsl],
                                 bc[:, 0, :])
            scrB = work.tile([64, 512], bf16, tag="scrB")
            nc.vector.tensor_mul(scrB[:], oUB[:, p, qsl], bc[:, 1, :])
            # partition shift 0-63 -> 64-127 (DVE lanes are partition-locked)
            nc.sync.dma_start(oTn2[64:128, p, qsl], scrB[:])

        def emit_oproj(rc):
            rsl = slice(rc * 128, (rc + 1) * 128)
            psy = ps_big.tile([128, 512], f32, tag="scores")
            for p in range(2):
                nc.tensor.matmul(psy[:], oTn2[:, p, rsl], wo[:, p, :],
                                 start=p == 0, stop=p == 1)
            yt = work.tile([128, 512], f32, tag="ysb")
            nc.vector.tensor_copy(yt[:], psy[:])
            nc.sync.dma_start(y_d[rsl, :], yt[:])

        # ---- main schedule --------------------------------------------------
        # attn(0,qc) carries p1's same-chunk qk burst + the previous chunk's
        # tail; attn(1,qc) carries p0's next qk burst + the next v chunk.
        def make_tail(qc, recs):
            out = [lambda p=p, qc=qc, r=recs[p]: emit_norm(p, qc, r)
                   for p in range(2)]
            out += [lambda rc=rc: emit_oproj(rc)
                    for rc in range(4 * qc, 4 * qc + 4)]
            return out

        emit_qk(0, 0)
        emit_v(0)
        recs = {}
        tails = {}
        for qc in range(NQC):
            inj0 = [lambda rc4=qc: emit_qk(1, rc4)]
            if qc == 0:
                inj0.append(lambda: emit_qk(0, 1))
            if qc - 1 in tails:
                inj0.extend(tails[qc - 1])
            recs[0] = emit_attn(0, qc, inj0)
            inj1 = []
            if qc == 0:
                inj1.append(lambda: emit_v(1))
            elif qc < NQC - 1:
                inj1.append(lambda rc4=qc + 1: emit_qk(0, rc4))
                inj1.append(lambda rc4=qc + 1: emit_v(rc4))
            if qc == NQC - 1:
                # last chunk: pair-0's normalize rides pair-1's attention so
                # only norm(1,3) + the final output projection trail the
                # last exp
                inj1.append(lambda r=recs[0]: emit_norm(0, qc, r))
            recs[1] = emit_attn(1, qc, inj1)
            tails[qc] = make_tail(qc, dict(recs))
        emit_norm(1, NQC - 1, recs[1])
        for rc in range(4 * (NQC - 1), 4 * NQC):
            emit_oproj(rc)

    nc.compile()
    return nc


def _diag_mask():
    i = np.arange(128)[None, :]
    j = np.arange(128)[:, None]
    return np.where(i >= j, 1.0, 0.0).astype(ml_dtypes.bfloat16)


def _prep_inputs(x, m, w_qkv, w_out):
    """Per-core input maps for SPMD dispatch."""
    dmS = _diag_mask()
    wq_full = w_qkv[:, 0:D]
    wk_full = w_qkv[:, D:2 * D]
    wv_full = w_qkv[:, 2 * D:3 * D]
    in_maps = []
    for c in range(8):
        b, q = c // 2, c % 2
        hsl = slice(4 * q * DH, (4 * q + 4) * DH)
        wq2 = np.stack([
            np.concatenate([wq_full[:, (4 * q + 2 * p) * DH:(4 * q + 2 * p + 1) * DH],
                            wq_full[:, (4 * q + 2 * p + 1) * DH:(4 * q + 2 * p + 2) * DH]],
                           axis=1)
            for p in range(2)])
        wk2 = np.stack([
            np.concatenate([wk_full[:, (4 * q + 2 * p) * DH:(4 * q + 2 * p + 1) * DH],
                            wk_full[:, (4 * q + 2 * p + 1) * DH:(4 * q + 2 * p + 2) * DH]],
                           axis=1)
            for p in range(2)])
        in_maps.append({
            "xT": np.ascontiguousarray(x[b].T).astype(ml_dtypes.bfloat16),
            "wq2": np.ascontiguousarray(wq2).astype(ml_dtypes.bfloat16),
            "wk2": np.ascontiguousarray(wk2).astype(ml_dtypes.bfloat16),
            "wv4": np.ascontiguousarray(wv_full[:, hsl]).astype(ml_dtypes.bfloat16),
            "wo4": np.ascontiguousarray(w_out[hsl, :]).astype(ml_dtypes.bfloat16),
            "dmS": dmS,
            "mkey": np.ascontiguousarray((m[b] != 0).astype(np.float32)[:, None]),
        })
    return in_maps


def _execute(inputs, trace=False):
    from concourse.bass_utils import run_bass_kernel_spmd

    if "nc" not in _CACHE:
        _CACHE["nc"] = _build_program()
    nc = _CACHE["nc"]

    x = np.asarray(inputs["x"], np.float32)
    m = np.asarray(inputs["m"], np.float32)
    w_qkv = np.asarray(inputs["w_qkv"], np.float32)
    w_out = np.asarray(inputs["w_out"], np.float32)
    b_out = np.asarray(inputs["b_out"], np.float32)

    in_maps = _prep_inputs(x, m, w_qkv, w_out)
    res = run_bass_kernel_spmd(nc, in_maps, core_ids=list(range(8)), trace=trace)

    y = np.empty((B, T, D), np.float32)
    for b in range(B):
        y[b] = res.results[2 * b]["y"] + res.results[2 * b + 1]["y"]
    y += b_out[None, None, :]
    y *= m[..., None]
    return y, res


def kernel(**inputs) -> np.ndarray:
    y, _ = _execute(inputs, trace=False)
    return y
